# revision 8
# baseline (speedup 1.0000x reference)
"""Trainium2 Bass kernel for NGCF-style embedding propagation (8 NeuronCores).

Math (reference, with A = adj / (sqrt(row_sum*col_sum)+eps)):
  updated_user = LReLU(A.T @ (item@W1) + (item * (A.T @ user)) @ W2 + user)
  updated_item = LReLU(A   @ (user@W1) + (user * (A   @ item)) @ W2 + item)

Row-shard adj across 8 cores (1024 rows each). Per core, with
Xr = s_r*[iown@W1, uown] (own rows) and Xc = s_c*[user@W1, item] (all cols):
  P^T = Xr^T @ adj  (per 512-col sub)   -> ReduceScatter over user blocks
  Q^T = xc^T @ adjT (accumulated)       -> local (own rows)

Design notes (vs the first working version, 224.4us -> 173.8us in the
TimelineSim cost model):
 - Q in transposed form: lhsT=xc (stationary), rhs=adjt block spanning 4
   row-blocks (512-wide moving) -> 128 Q matmuls instead of 512, removing
   PE-sequencer pressure (the old kernel was queue-backpressure bound).
 - ei = [user@W1 | item] built from OWN rows only (uown@W1, iown already
   loaded per-core) and AllGathered as fp16 with 2KB descriptors, replacing
   23us of 256B-descriptor full-embedding DMA with ~7us.
 - Row sums via free-size-1 matmuls on the transposed blocks (engine-free
   in the cost model) instead of Act accum_out; col+row sums share one
   PSUM bank primed ONCE by a zero-matmul (start=True) with all later
   matmuls start=False — interleaved per-region start flags silently
   break cross-panel PSUM accumulation.
 - Per-panel colsum AllReduce roundtrip takes >1 panel (SWDGE gen + 3x
   900ns DMA sem props), so its consumption (sqrt/recip -> xc -> Q) is
   lagged TWO panels and emitted just before that panel's first adjT
   write; emitting it earlier stalls the in-order Act/DVE queues at the
   head and starves the casts.
 - Adjacency streamed as [128,512] fp32 half-chunks, 5-deep: the
   DMA->sem->cast->SEQ->HWDGE->DGE release chain is ~2.9us, so 4 buffers
   sit exactly at the 728ns/transfer cadence and any jitter stalls DMA.
 - item/user finish in transposed [feat, row] space: (q1*uown)@W2
   accumulates straight onto q0 in PSUM (walrus rejects dual-PSUM-operand
   DVE ops), then 8 small output transposes per side. W1/W2/ownT
   embeddings are mirrored on partitions 64-127 so the upper (q1/P1)
   halves stay partition-aligned.
 - P split even/odd subs into two ReduceScatters so the first readback +
   user finish overlap the second half of the P stream.
"""

import numpy as np

N = 8192
D = 64
NCORES = 8
U = N // NCORES          # rows per core = 1024
UB = U // 128            # 128-row blocks per core = 8
CB = N // 128            # 128-col blocks = 64
PAN = 8                  # column panels
PCB = CB // PAN          # col blocks per panel = 8
PW = PCB * 128           # panel width = 1024

_CACHE = {}


def _build(dbg=False, single=False):
    import concourse.bass as bass
    import concourse.bacc as bacc
    import concourse.mybir as mybir
    import concourse.tile as tile
    from concourse import masks

    f32 = mybir.dt.float32
    f16 = mybir.dt.float16
    AF = mybir.ActivationFunctionType
    ALU = mybir.AluOpType
    ds = bass.ds

    nc = bacc.Bacc("TRN2", target_bir_lowering=False, debug=False,
                   num_devices=(1 if single else NCORES), enable_asserts=False)

    adj = nc.dram_tensor("adj", [U, N], f32, kind="ExternalInput").ap()
    user_own = nc.dram_tensor("user_own", [U, D], f32, kind="ExternalInput").ap()
    item_own = nc.dram_tensor("item_own", [U, D], f32, kind="ExternalInput").ap()
    w1 = nc.dram_tensor("w1", [D, D], f32, kind="ExternalInput").ap()
    w2 = nc.dram_tensor("w2", [D, D], f32, kind="ExternalInput").ap()
    upd_user = nc.dram_tensor("upd_user", [U, D], f32, kind="ExternalOutput").ap()
    upd_item = nc.dram_tensor("upd_item", [U, D], f32, kind="ExternalOutput").ap()

    groups = [list(range(NCORES))]

    with tile.TileContext(nc) as tc:
        with (
            tc.tile_pool(name="persist", bufs=1) as persist,
            tc.tile_pool(name="ld", bufs=5) as ldp,
            tc.tile_pool(name="small", bufs=2) as small,
            tc.tile_pool(name="fin", bufs=1) as fin,
            tc.tile_pool(name="pstp", bufs=3) as pstp,
            tc.tile_pool(name="ps_sm", bufs=1, space="PSUM") as ps_sm,
            tc.tile_pool(name="ps2k", bufs=4, space="PSUM") as ps2k,
            tc.tile_pool(name="ps_q", bufs=1, space="PSUM") as ps_q,
            tc.tile_pool(name="dram", bufs=1, space="DRAM") as dram,
        ):
            # ---------------- persistent SBUF tiles
            cache = persist.tile([128, UB, N], f16)          # 128 KiB/part
            adjt = persist.tile([128, 2, PCB, UB, 128], f16)  # 32 KiB
            ei = persist.tile([128, CB, 2 * D], f16)         # 16 KiB (e1|item)
            xc = persist.tile([128, PCB, 2 * D], f16)        # 2 KiB
            uown = persist.tile([128, UB, D], f16)           # 1
            iown = persist.tile([128, UB, D], f16)           # 1
            uown_t = persist.tile([128, UB, 128], f16)       # 2 (parts 64:128)
            iown_t = persist.tile([128, UB, 128], f16)       # 2 (parts 64:128)
            xr = persist.tile([128, UB, 2 * D], f16)         # 2 (x0r then Xr)
            eist = persist.tile([128, UB, 2 * D], f16)       # 2 ([uW1|iown] own)

            s_r = persist.tile([128, UB], f32)
            s_c = persist.tile([128, CB], f32)
            out_stage = persist.tile([128, UB, D], f16)      # 1
            w1_hi = persist.tile([128, D], f16)              # parts 64:128
            w2_hi = persist.tile([128, D], f16)              # parts 64:128
            ones_hf = persist.tile([128, 1], f16)
            onerow = persist.tile([1, 128], f16)
            zrow = persist.tile([1, CB + UB], f16)
            ident = persist.tile([128, 128], f16)
            pt_sb = eist                                     # reuse (dead then)

            psum_qt = ps_q.tile([128, UB, 128], f32)         # 2 banks, Q^T
            psum_cr = ps_q.tile([128, CB + UB], f32)         # col+row sums

            nc.gpsimd.memset(ones_hf[:], 1.0)
            nc.gpsimd.memset(onerow[:], 1.0)
            nc.gpsimd.memset(zrow[:], 0.0)
            masks.make_identity(nc, ident[:])
            # prime psum_cr: one zero-matmul start=True opens a single
            # accumulation group for every col/row-sum region; all later
            # free-size-1 matmuls accumulate with start=False
            nc.tensor.matmul(psum_cr[:], onerow[:], zrow[:],
                             start=True, stop=False, skip_group_check=True)

            # W1/W2 -> fp16 on partitions 64:128 (stationary for the
            # transposed-space matmuls whose moving operand sits there)
            for wsrc, wdst in ((w1, w1_hi), (w2, w2_hi)):
                wld = small.tile([128, D], f32, tag="wld")
                nc.gpsimd.dma_start(wld[64:128], wsrc)
                nc.vector.tensor_copy(wdst[64:128], wld[64:128])

            # own embeddings (fp32 in DRAM, fp16 in SBUF via DMA convert)
            uo_view = user_own.rearrange("(ub p) d -> p ub d", p=128)
            io_view = item_own.rearrange("(ub p) d -> p ub d", p=128)
            nc.gpsimd.dma_start(uown[:], uo_view)
            nc.gpsimd.dma_start(iown[:], io_view)

            # setup compute, emitted at panel-0 hooks so the early adjacency
            # casts aren't queued behind it
            def setup_ownt():
                # transposed own embeddings on partitions 64:128
                for ub in range(UB):
                    pt = ps_sm.tile([128, 2, 128], f16, tag="sm")
                    nc.tensor.transpose(pt[64:128, 0], uown[:, ub], ident[:])
                    nc.tensor.transpose(pt[64:128, 1], iown[:, ub], ident[:])
                    nc.vector.tensor_copy(uown_t[64:128, ub], pt[64:128, 0])
                    nc.vector.tensor_copy(iown_t[64:128, ub], pt[64:128, 1])

            def setup_eist(ubs):
                # eist = [uown@W1 | iown]; x0r = [iown@W1 | uown]
                for ub in ubs:
                    pe = ps_sm.tile([128, 2, D], f32, tag="sm")
                    nc.tensor.matmul(pe[:, 0], uown_t[64:128, ub],
                                     w1_hi[64:128], start=True, stop=True)
                    nc.tensor.matmul(pe[:, 1], iown_t[64:128, ub],
                                     w1_hi[64:128], start=True, stop=True)
                    nc.scalar.activation(eist[:, ub, 0:D], pe[:, 0], AF.Copy)
                    nc.scalar.activation(xr[:, ub, 0:D], pe[:, 1], AF.Copy)
                if ubs[-1] == UB - 1:
                    nc.vector.tensor_copy(
                        eist[:].rearrange(
                            "p ub (h d) -> p ub h d", h=2)[:, :, 1],
                        iown[:])
                    nc.vector.tensor_copy(
                        xr[:].rearrange(
                            "p ub (h d) -> p ub h d", h=2)[:, :, 1],
                        uown[:])

            # AllGather ei = [user@W1 | item] as fp16 (2KB descriptors)
            ei_in = dram.tile([128, UB, 2 * D], f16, name="ei_in")
            ei_ag = dram.tile([NCORES, 128, UB, 2 * D], f16,
                              addr_space="Shared", name="ei_ag")

            def setup_ag():
                nc.gpsimd.dma_start(ei_in[:], eist[:])
                if single:
                    nc.gpsimd.dma_start(ei_ag[0], ei_in[:])
                else:
                    nc.gpsimd.collective_compute(
                        "AllGather", mybir.AluOpType.bypass,
                        replica_groups=groups,
                        ins=[ei_in.opt()], outs=[ei_ag.opt()])
                nc.gpsimd.dma_start(
                    ei[:].rearrange("p (g ub) f -> p g ub f", g=NCORES),
                    ei_ag.rearrange("g p ub f -> p g ub f"))

            # per-panel column-sum AllReduce buffers
            col_in = []
            col_out = []
            for _pn in range(PAN):
                ci = dram.tile([128, PCB], f32, name=f"col_in{_pn}")
                co = dram.tile([128, PCB], f32, addr_space="Shared",
                               name=f"col_out{_pn}")
                col_in.append(ci)
                col_out.append(co)

            adj_v = adj.rearrange("(ub p) n -> p ub n", p=128)

            def emit_q(panel, j0=0, j1=PCB):
                """Q^T matmuls for a completed panel (lagged): accumulate
                [2D, (ub r)] with 512-wide moving operands."""
                buf = panel % 2
                for j in range(j0, j1):
                    st = (panel == 0 and j == 0)
                    sp = False
                    nc.tensor.matmul(psum_qt[:, 0:4], xc[:, j],
                                     adjt[:, buf, j, 0:4],
                                     start=st, stop=sp, skip_group_check=True)
                    nc.tensor.matmul(psum_qt[:, 4:8], xc[:, j],
                                     adjt[:, buf, j, 4:8],
                                     start=st, stop=sp, skip_group_check=True)

            colsb2 = [None] * PAN

            def emit_sc_xc(panel):
                """s_c + Xc for a panel whose AllReduce result is back."""
                csl = slice(panel * PCB, (panel + 1) * PCB)
                sqc = small.tile([128, PCB], f32, tag="sqc")
                nc.scalar.sqrt(sqc[:], colsb2[panel][:])
                nc.vector.reciprocal(s_c[:, csl], sqc[:])
                for j in range(PCB):
                    cb = panel * PCB + j
                    nc.vector.tensor_scalar(
                        xc[:, j], ei[:, cb], s_c[:, cb:cb + 1],
                        None, ALU.mult)

            def emit_rowsums(pan, ub):
                """Row-sum partials from the transposed blocks: free-size-1
                matmuls (engine-free); lagged 2 chunks so the adjT staging
                copy is guaranteed done."""
                buf = pan % 2
                for j in range(PCB):
                    nc.tensor.matmul(
                        psum_cr[:, CB + ub:CB + ub + 1],
                        adjt[:, buf, j, ub], ones_hf[:],
                        start=False,
                        stop=(pan == PAN - 1 and j == PCB - 1),
                        skip_group_check=True)

            # ---------------- phase A: panel-major streaming
            chunk_hist = []
            for panel in range(PAN):
                for ub in range(UB):
                    # lag-2 panel chain: the AllReduce roundtrip takes more
                    # than one panel on the Pool SWDGE queue, so consume its
                    # result (s_c -> xc -> Q) two panels later, just before
                    # this panel's first adjT write (order guards the WAR)
                    if ub == 0 and panel >= 2:
                        emit_sc_xc(panel - 2)
                        emit_q(panel - 2)
                    chunk_hist.append((panel, ub))
                    if len(chunk_hist) > 2:
                        emit_rowsums(*chunk_hist[-3])
                    pst = ps2k.tile([128, PCB, 128], f16, tag="s2k")
                    for half in range(2):
                        hw = PW // 2
                        c0h = panel * PW + half * hw
                        ld = ldp.tile([128, hw], f32, tag="ld")
                        nc.sync.dma_start(ld[:], adj_v[:, ub, c0h:c0h + hw])
                        nc.scalar.activation(
                            cache[:, ub, c0h:c0h + hw], ld[:], AF.Copy)
                        # PE transposes -> PSUM (fp16), staged to adjT on DVE
                        for jh in range(PCB // 2):
                            j = half * (PCB // 2) + jh
                            c0 = panel * PW + j * 128
                            nc.tensor.transpose(pst[:, j],
                                                cache[:, ub, c0:c0 + 128],
                                                ident[:])
                        # column partial sums (free-size-1 matmuls: ~0 engine)
                        for jh in range(PCB // 2):
                            j = half * (PCB // 2) + jh
                            cb = panel * PCB + j
                            c0 = cb * 128
                            nc.tensor.matmul(
                                psum_cr[:, cb:cb + 1],
                                cache[:, ub, c0:c0 + 128], ones_hf[:],
                                start=False,
                                stop=(panel == PAN - 1 and ub == UB - 1),
                                skip_group_check=True)
                    nc.vector.tensor_copy(adjt[:, panel % 2, :, ub], pst[:])
                    # setup compute hooks (panel 0) and the lagged per-panel
                    # chain, emitted late enough that the AllReduce is
                    # already back (no queue-head stall)
                    if panel == 0:
                        if ub == 1:
                            setup_ownt()
                        elif ub == 2:
                            setup_eist(list(range(4)))
                        elif ub == 3:
                            setup_eist(list(range(4, UB)))
                        elif ub == 4:
                            setup_ag()

                # panel column sums complete -> AllReduce (latency hidden)
                csl = slice(panel * PCB, (panel + 1) * PCB)
                col_sb = small.tile([128, PCB], f32, tag="colsb")
                nc.vector.tensor_copy(col_sb[:], psum_cr[:, csl])
                nc.scalar.dma_start(col_in[panel][:], col_sb[:])
                if single:
                    nc.gpsimd.dma_start(col_out[panel][:], col_in[panel][:])
                else:
                    nc.gpsimd.collective_compute(
                        "AllReduce", mybir.AluOpType.add, replica_groups=groups,
                        ins=[col_in[panel].opt()], outs=[col_out[panel].opt()])
                cb2 = small.tile([128, PCB], f32, tag="cs2", name=f"cs2_{panel}")
                colsb2[panel] = cb2
                nc.gpsimd.dma_start(cb2[:], col_out[panel][:])

            # ---------------- tail
            emit_rowsums(*chunk_hist[-2])
            emit_rowsums(*chunk_hist[-1])

            # s_r and Xr (scale x0r in place)
            sqr = small.tile([128, UB], f32, tag="sqr2")
            nc.scalar.sqrt(sqr[:], psum_cr[:, CB:CB + UB])
            nc.vector.reciprocal(s_r[:], sqr[:])
            for ub in range(UB):
                nc.scalar.activation(xr[:, ub], xr[:, ub], AF.Copy,
                                     scale=s_r[:, ub:ub + 1])

            def emit_item_finish():
                """out_item = LReLU(s_r*(q0 + (q1*uown)@W2) + iown), done in
                transposed space: q0T/q1T = psum_qt[0:64]/[64:128]."""
                for h in range(2):
                    hsl = slice(4 * h, 4 * (h + 1))
                    g = fin.tile([128, 4, 128], f16, tag="g")
                    nc.vector.tensor_mul(g[64:128], psum_qt[64:128, hsl],
                                         uown_t[64:128, hsl])
                    # accumulate (q1*uown)@W2 directly onto q0T in PSUM
                    nc.tensor.matmul(psum_qt[0:64, hsl], w2_hi[64:128],
                                     g[64:128], start=False, stop=True,
                                     skip_group_check=True)
                    sh = fin.tile([64, 4, 128], f16, tag="g")
                    nc.vector.tensor_copy(sh[:], psum_qt[0:64, hsl])
                    tr_ps = ps_sm.tile([128, 4, D], f16, tag="sm")
                    for k in range(4):
                        ub = 4 * h + k
                        nc.tensor.transpose(tr_ps[:, k], sh[:, k],
                                            ident[0:64, 0:64])
                    for k in range(4):
                        ub = 4 * h + k
                        tb = small.tile([128, D], f32, tag="ft")
                        nc.vector.scalar_tensor_tensor(
                            tb[:], tr_ps[:, k], s_r[:, ub:ub + 1],
                            iown[:, ub], ALU.mult, ALU.add)
                        nc.vector.scalar_tensor_tensor(
                            out_stage[:, ub], tb[:], 0.2, tb[:],
                            ALU.mult, ALU.max)
                ui_view = upd_item.rearrange("(ub p) d -> p ub d", p=128)
                nc.gpsimd.dma_start(ui_view[:], out_stage[:])

            # P^T: stationary Xr[ub], moving natural cache; 512B-desc pairs.
            # Even subs feed p_in_a (each core's pairs 4g,4g+1 = ub 0-3),
            # odd subs feed p_in_b (ub 4-7); evens run first so the first
            # ReduceScatter + readback + user finish overlap the odd half.
            p_in_a = dram.tile([NCORES, 2, 128, 256], f16, name="p_in_a")
            p_in_b = dram.tile([NCORES, 2, 128, 256], f16, name="p_in_b")
            p_out_a = dram.tile([2, 128, 256], f16, name="p_out_a")
            p_out_b = dram.tile([2, 128, 256], f16, name="p_out_b")

            def emit_rs(p_in_t, p_out_t):
                if single:
                    nc.sync.dma_start(p_out_t[:], p_in_t[0])
                else:
                    nc.gpsimd.collective_compute(
                        "ReduceScatter", mybir.AluOpType.add,
                        replica_groups=groups,
                        ins=[p_in_t.opt()], outs=[p_out_t.opt()])

            pid = nc.vector.partition_id()
            uu_view = upd_user.rearrange("(ub p) d -> p ub d", p=128)

            def finish_user(h):
                """out_user = LReLU(s_c*(P0 + (P1*iown)@W2) + uown), half h,
                in transposed space directly from the ReduceScatter output."""
                hsl = slice(4 * h, 4 * (h + 1))
                p_out_t = p_out_a if h == 0 else p_out_b
                nc.sync.dma_start(
                    pt_sb[:, hsl].rearrange("p (b x) c -> p b (x c)", x=2),
                    p_out_t.rearrange("b d c -> d b c"))
                g2 = fin.tile([128, 4, 128], f16, tag="g")
                nc.vector.tensor_mul(g2[64:128],
                                     pt_sb[64:128, hsl],
                                     iown_t[64:128, hsl])
                ph2 = ps2k.tile([64, 4, 128], f32, tag="s2k")
                nc.tensor.matmul(ph2[:], w2_hi[64:128], g2[64:128],
                                 start=True, stop=True)
                sh2 = fin.tile([64, 4, 128], f16, tag="g")
                nc.vector.scalar_tensor_tensor(
                    sh2[:], pt_sb[0:64, hsl], 1.0, ph2[:], ALU.mult, ALU.add)
                tr2 = ps_sm.tile([128, 4, D], f16, tag="sm")
                for k in range(4):
                    nc.tensor.transpose(tr2[:, k], sh2[:, k],
                                        ident[0:64, 0:64])
                for k in range(4):
                    ub = 4 * h + k
                    if single:
                        sc_ap = s_c[:, ub:ub + 1]
                    else:
                        sc_ap = s_c[:, ds(pid * UB + ub, 1)]
                    t1 = small.tile([128, D], f32, tag="ft")
                    nc.vector.scalar_tensor_tensor(
                        t1[:], tr2[:, k], sc_ap, uown[:, ub],
                        ALU.mult, ALU.add)
                    nc.vector.scalar_tensor_tensor(
                        out_stage[:, ub], t1[:], 0.2, t1[:],
                        ALU.mult, ALU.max)
                nc.gpsimd.dma_start(uu_view[:, hsl], out_stage[:, hsl])

            sub_order = [2 * t for t in range(8)] + [2 * t + 1 for t in range(8)]
            for t, sub in enumerate(sub_order):
                pp = ps2k.tile([128, 512], f32, tag="s2k")
                for ub in range(UB):
                    nc.tensor.matmul(
                        pp[:], xr[:, ub], cache[:, ub, sub * 512:(sub + 1) * 512],
                        start=(ub == 0), stop=(ub == UB - 1),
                        skip_group_check=True)
                pcast = pstp.tile([128, 2, 256], f16, tag="pst")
                if t % 2:
                    nc.vector.tensor_copy(pcast[:], pp[:])
                else:
                    nc.scalar.activation(pcast[:], pp[:], AF.Copy)
                p_in_t = p_in_a if sub % 2 == 0 else p_in_b
                nc.sync.dma_start(
                    p_in_t[sub // 2].rearrange("b d c -> d b c"), pcast[:])
                if t == 0:
                    emit_sc_xc(PAN - 2)
                    emit_q(PAN - 2, 0, 4)
                elif t == 1:
                    emit_q(PAN - 2, 4, 8)
                elif t == 4:
                    emit_sc_xc(PAN - 1)
                elif t == 5:
                    emit_q(PAN - 1, 0, 4)
                elif t == 6:
                    emit_q(PAN - 1, 4, 8)
                elif t == 7:
                    emit_rs(p_in_a, p_out_a)
                elif t == 8:
                    emit_item_finish()
                elif t == 11:
                    finish_user(0)
            emit_rs(p_in_b, p_out_b)
            finish_user(1)

    nc.compile()
    return nc


def _get_nc(dbg=False):
    key = ("nc", dbg)
    if key not in _CACHE:
        _CACHE[key] = _build(dbg)
    return _CACHE[key]


def make_in_maps(user_embeddings, item_embeddings, adjacency_matrix, W1, W2):
    adj = np.ascontiguousarray(np.asarray(adjacency_matrix, dtype=np.float32))
    ue = np.ascontiguousarray(np.asarray(user_embeddings, dtype=np.float32))
    ie = np.ascontiguousarray(np.asarray(item_embeddings, dtype=np.float32))
    w1 = np.ascontiguousarray(np.asarray(W1, dtype=np.float32))
    w2 = np.ascontiguousarray(np.asarray(W2, dtype=np.float32))
    in_maps = []
    for k in range(NCORES):
        sl = slice(k * U, (k + 1) * U)
        in_maps.append({
            "adj": np.ascontiguousarray(adj[sl]),
            "user_own": np.ascontiguousarray(ue[sl]),
            "item_own": np.ascontiguousarray(ie[sl]),
            "w1": w1,
            "w2": w2,
        })
    return in_maps


def assemble(results):
    upd_user = np.concatenate([results[k]["upd_user"] for k in range(NCORES)], 0)
    upd_item = np.concatenate([results[k]["upd_item"] for k in range(NCORES)], 0)
    return upd_user, upd_item


def kernel(user_embeddings, item_embeddings, adjacency_matrix, W1, W2):
    import time
    import concourse.bass_utils as bass_utils
    nc = _get_nc()
    in_maps = make_in_maps(user_embeddings, item_embeddings, adjacency_matrix,
                           W1, W2)
    last = None
    for attempt in range(3):
        try:
            res = bass_utils.run_bass_kernel_spmd(
                nc, in_maps, core_ids=list(range(NCORES)), trace=False)
            return assemble(res.results)
        except Exception as e:  # transient NRT/axon failures
            last = e
            time.sleep(10)
    raise last


# revision 9
# speedup vs baseline: 1.0013x; 1.0013x over previous
"""Trainium2 Bass kernel for NGCF-style embedding propagation (8 NeuronCores).

Math (reference, with A = adj / (sqrt(row_sum*col_sum)+eps)):
  updated_user = LReLU(A.T @ (item@W1) + (item * (A.T @ user)) @ W2 + user)
  updated_item = LReLU(A   @ (user@W1) + (user * (A   @ item)) @ W2 + item)

Row-shard adj across 8 cores (1024 rows each). Per core, with
Xr = s_r*[iown@W1, uown] (own rows) and Xc = s_c*[user@W1, item] (all cols):
  P^T = Xr^T @ adj  (per 512-col sub)   -> ReduceScatter over user blocks
  Q^T = xc^T @ adjT (accumulated)       -> local (own rows)

Design notes (vs the first working version, 224.4us -> 173.8us in the
TimelineSim cost model):
 - Q in transposed form: lhsT=xc (stationary), rhs=adjt block spanning 4
   row-blocks (512-wide moving) -> 128 Q matmuls instead of 512, removing
   PE-sequencer pressure (the old kernel was queue-backpressure bound).
 - ei = [user@W1 | item] built from OWN rows only (uown@W1, iown already
   loaded per-core) and AllGathered as fp16 with 2KB descriptors, replacing
   23us of 256B-descriptor full-embedding DMA with ~7us.
 - Row sums via free-size-1 matmuls on the transposed blocks (engine-free
   in the cost model) instead of Act accum_out; col+row sums share one
   PSUM bank primed ONCE by a zero-matmul (start=True) with all later
   matmuls start=False — interleaved per-region start flags silently
   break cross-panel PSUM accumulation.
 - Per-panel colsum AllReduce roundtrip takes >1 panel (SWDGE gen + 3x
   900ns DMA sem props), so its consumption (sqrt/recip -> xc -> Q) is
   lagged TWO panels and emitted just before that panel's first adjT
   write; emitting it earlier stalls the in-order Act/DVE queues at the
   head and starves the casts.
 - Adjacency streamed as [128,512] fp32 half-chunks, 5-deep: the
   DMA->sem->cast->SEQ->HWDGE->DGE release chain is ~2.9us, so 4 buffers
   sit exactly at the 728ns/transfer cadence and any jitter stalls DMA.
 - item/user finish in transposed [feat, row] space: (q1*uown)@W2
   accumulates straight onto q0 in PSUM (walrus rejects dual-PSUM-operand
   DVE ops), then 8 small output transposes per side. W1/W2/ownT
   embeddings are mirrored on partitions 64-127 so the upper (q1/P1)
   halves stay partition-aligned.
 - P split even/odd subs into two ReduceScatters so the first readback +
   user finish overlap the second half of the P stream.
"""

import numpy as np

N = 8192
D = 64
NCORES = 8
U = N // NCORES          # rows per core = 1024
UB = U // 128            # 128-row blocks per core = 8
CB = N // 128            # 128-col blocks = 64
PAN = 8                  # column panels
PCB = CB // PAN          # col blocks per panel = 8
PW = PCB * 128           # panel width = 1024

_CACHE = {}


def _build(dbg=False, single=False):
    import concourse.bass as bass
    import concourse.bacc as bacc
    import concourse.mybir as mybir
    import concourse.tile as tile
    from concourse import masks

    f32 = mybir.dt.float32
    f16 = mybir.dt.float16
    AF = mybir.ActivationFunctionType
    ALU = mybir.AluOpType
    ds = bass.ds

    nc = bacc.Bacc("TRN2", target_bir_lowering=False, debug=False,
                   num_devices=(1 if single else NCORES), enable_asserts=False)

    adj = nc.dram_tensor("adj", [U, N], f32, kind="ExternalInput").ap()
    user_own = nc.dram_tensor("user_own", [U, D], f32, kind="ExternalInput").ap()
    item_own = nc.dram_tensor("item_own", [U, D], f32, kind="ExternalInput").ap()
    w1 = nc.dram_tensor("w1", [D, D], f32, kind="ExternalInput").ap()
    w2 = nc.dram_tensor("w2", [D, D], f32, kind="ExternalInput").ap()
    upd_user = nc.dram_tensor("upd_user", [U, D], f32, kind="ExternalOutput").ap()
    upd_item = nc.dram_tensor("upd_item", [U, D], f32, kind="ExternalOutput").ap()

    groups = [list(range(NCORES))]

    with tile.TileContext(nc) as tc:
        with (
            tc.tile_pool(name="persist", bufs=1) as persist,
            tc.tile_pool(name="ld", bufs=5) as ldp,
            tc.tile_pool(name="small", bufs=2) as small,
            tc.tile_pool(name="fin", bufs=1) as fin,
            tc.tile_pool(name="pstp", bufs=3) as pstp,
            tc.tile_pool(name="ps_sm", bufs=1, space="PSUM") as ps_sm,
            tc.tile_pool(name="ps2k", bufs=4, space="PSUM") as ps2k,
            tc.tile_pool(name="ps_q", bufs=1, space="PSUM") as ps_q,
            tc.tile_pool(name="dram", bufs=1, space="DRAM") as dram,
        ):
            # ---------------- persistent SBUF tiles
            cache = persist.tile([128, UB, N], f16)          # 128 KiB/part
            adjt = persist.tile([128, 2, PCB, UB, 128], f16)  # 32 KiB
            ei = persist.tile([128, CB, 2 * D], f16)         # 16 KiB (e1|item)
            xc = persist.tile([128, PCB, 2 * D], f16)        # 2 KiB
            uown = persist.tile([128, UB, D], f16)           # 1
            iown = persist.tile([128, UB, D], f16)           # 1
            uown_t = persist.tile([128, UB, 128], f16)       # 2 (parts 64:128)
            iown_t = persist.tile([128, UB, 128], f16)       # 2 (parts 64:128)
            xr = persist.tile([128, UB, 2 * D], f16)         # 2 (x0r then Xr)
            eist = persist.tile([128, UB, 2 * D], f16)       # 2 ([uW1|iown] own)

            s_r = persist.tile([128, UB], f32)
            s_c = persist.tile([128, CB], f32)
            out_stage = persist.tile([128, UB, D], f16)      # 1
            w1_hi = persist.tile([128, D], f16)              # parts 64:128
            w2_hi = persist.tile([128, D], f16)              # parts 64:128
            ones_hf = persist.tile([128, 1], f16)
            onerow = persist.tile([1, 128], f16)
            zrow = persist.tile([1, CB + UB], f16)
            ident = persist.tile([128, 128], f16)
            pt_sb = eist                                     # reuse (dead then)

            psum_qt = ps_q.tile([128, UB, 128], f32)         # 2 banks, Q^T
            psum_cr = ps_q.tile([128, CB + UB], f32)         # col+row sums

            nc.gpsimd.memset(ones_hf[:], 1.0)
            nc.gpsimd.memset(onerow[:], 1.0)
            nc.gpsimd.memset(zrow[:], 0.0)
            masks.make_identity(nc, ident[:])
            # prime psum_cr: one zero-matmul start=True opens a single
            # accumulation group for every col/row-sum region; all later
            # free-size-1 matmuls accumulate with start=False
            nc.tensor.matmul(psum_cr[:], onerow[:], zrow[:],
                             start=True, stop=False, skip_group_check=True)

            # W1/W2 -> fp16 on partitions 64:128 (stationary for the
            # transposed-space matmuls whose moving operand sits there)
            for wsrc, wdst in ((w1, w1_hi), (w2, w2_hi)):
                wld = small.tile([128, D], f32, tag="ft")
                nc.gpsimd.dma_start(wld[64:128], wsrc)
                nc.vector.tensor_copy(wdst[64:128], wld[64:128])

            # own embeddings (fp32 in DRAM, fp16 in SBUF via DMA convert)
            uo_view = user_own.rearrange("(ub p) d -> p ub d", p=128)
            io_view = item_own.rearrange("(ub p) d -> p ub d", p=128)
            nc.gpsimd.dma_start(uown[:], uo_view)
            nc.gpsimd.dma_start(iown[:], io_view)

            # setup compute, emitted at panel-0 hooks so the early adjacency
            # casts aren't queued behind it
            def setup_ownt():
                # transposed own embeddings on partitions 64:128
                for ub in range(UB):
                    pt = ps_sm.tile([128, 2, 128], f16, tag="sm")
                    nc.tensor.transpose(pt[64:128, 0], uown[:, ub], ident[:])
                    nc.tensor.transpose(pt[64:128, 1], iown[:, ub], ident[:])
                    nc.vector.tensor_copy(uown_t[64:128, ub], pt[64:128, 0])
                    nc.vector.tensor_copy(iown_t[64:128, ub], pt[64:128, 1])

            def setup_eist(ubs):
                # eist = [uown@W1 | iown]; x0r = [iown@W1 | uown]
                for ub in ubs:
                    pe = ps_sm.tile([128, 2, D], f32, tag="sm")
                    nc.tensor.matmul(pe[:, 0], uown_t[64:128, ub],
                                     w1_hi[64:128], start=True, stop=True)
                    nc.tensor.matmul(pe[:, 1], iown_t[64:128, ub],
                                     w1_hi[64:128], start=True, stop=True)
                    nc.scalar.activation(eist[:, ub, 0:D], pe[:, 0], AF.Copy)
                    nc.scalar.activation(xr[:, ub, 0:D], pe[:, 1], AF.Copy)
                if ubs[-1] == UB - 1:
                    nc.vector.tensor_copy(
                        eist[:].rearrange(
                            "p ub (h d) -> p ub h d", h=2)[:, :, 1],
                        iown[:])
                    nc.vector.tensor_copy(
                        xr[:].rearrange(
                            "p ub (h d) -> p ub h d", h=2)[:, :, 1],
                        uown[:])

            # AllGather ei = [user@W1 | item] as fp16 (2KB descriptors)
            ei_in = dram.tile([128, UB, 2 * D], f16, name="ei_in")
            ei_ag = dram.tile([NCORES, 128, UB, 2 * D], f16,
                              addr_space="Shared", name="ei_ag")

            def setup_ag():
                nc.gpsimd.dma_start(ei_in[:], eist[:])
                if single:
                    nc.gpsimd.dma_start(ei_ag[0], ei_in[:])
                else:
                    nc.gpsimd.collective_compute(
                        "AllGather", mybir.AluOpType.bypass,
                        replica_groups=groups,
                        ins=[ei_in.opt()], outs=[ei_ag.opt()])
                nc.gpsimd.dma_start(
                    ei[:].rearrange("p (g ub) f -> p g ub f", g=NCORES),
                    ei_ag.rearrange("g p ub f -> p g ub f"))

            # per-panel column-sum AllReduce buffers
            col_in = []
            col_out = []
            for _pn in range(PAN):
                ci = dram.tile([128, PCB], f32, name=f"col_in{_pn}")
                co = dram.tile([128, PCB], f32, addr_space="Shared",
                               name=f"col_out{_pn}")
                col_in.append(ci)
                col_out.append(co)

            adj_v = adj.rearrange("(ub p) n -> p ub n", p=128)

            def emit_q(panel, j0=0, j1=PCB):
                """Q^T matmuls for a completed panel (lagged): accumulate
                [2D, (ub r)] with 512-wide moving operands."""
                buf = panel % 2
                for j in range(j0, j1):
                    st = (panel == 0 and j == 0)
                    sp = False
                    nc.tensor.matmul(psum_qt[:, 0:4], xc[:, j],
                                     adjt[:, buf, j, 0:4],
                                     start=st, stop=sp, skip_group_check=True)
                    nc.tensor.matmul(psum_qt[:, 4:8], xc[:, j],
                                     adjt[:, buf, j, 4:8],
                                     start=st, stop=sp, skip_group_check=True)

            colsb2 = [None] * PAN

            def emit_sc_xc(panel):
                """s_c + Xc for a panel whose AllReduce result is back."""
                csl = slice(panel * PCB, (panel + 1) * PCB)
                sqc = small.tile([128, PCB], f32, tag="sqc")
                nc.scalar.sqrt(sqc[:], colsb2[panel][:])
                nc.vector.reciprocal(s_c[:, csl], sqc[:])
                for j in range(PCB):
                    cb = panel * PCB + j
                    nc.vector.tensor_scalar(
                        xc[:, j], ei[:, cb], s_c[:, cb:cb + 1],
                        None, ALU.mult)

            def emit_rowsums(pan, ub):
                """Row-sum partials from the transposed blocks: free-size-1
                matmuls (engine-free); lagged 2 chunks so the adjT staging
                copy is guaranteed done."""
                buf = pan % 2
                for j in range(PCB):
                    nc.tensor.matmul(
                        psum_cr[:, CB + ub:CB + ub + 1],
                        adjt[:, buf, j, ub], ones_hf[:],
                        start=False,
                        stop=(pan == PAN - 1 and j == PCB - 1),
                        skip_group_check=True)

            # ---------------- phase A: panel-major streaming
            chunk_hist = []
            for panel in range(PAN):
                for ub in range(UB):
                    # lag-2 panel chain: the AllReduce roundtrip takes more
                    # than one panel on the Pool SWDGE queue, so consume its
                    # result (s_c -> xc -> Q) two panels later, just before
                    # this panel's first adjT write (order guards the WAR)
                    if ub == 0 and panel >= 2:
                        emit_sc_xc(panel - 2)
                        emit_q(panel - 2)
                    chunk_hist.append((panel, ub))
                    if len(chunk_hist) > 2:
                        emit_rowsums(*chunk_hist[-3])
                    pst = ps2k.tile([128, PCB, 128], f16, tag="s2k")
                    for half in range(2):
                        hw = PW // 2
                        c0h = panel * PW + half * hw
                        ld = ldp.tile([128, hw], f32, tag="ld")
                        nc.sync.dma_start(ld[:], adj_v[:, ub, c0h:c0h + hw])
                        nc.scalar.activation(
                            cache[:, ub, c0h:c0h + hw], ld[:], AF.Copy)
                        # PE transposes -> PSUM (fp16), staged to adjT on DVE
                        for jh in range(PCB // 2):
                            j = half * (PCB // 2) + jh
                            c0 = panel * PW + j * 128
                            nc.tensor.transpose(pst[:, j],
                                                cache[:, ub, c0:c0 + 128],
                                                ident[:])
                        # column partial sums (free-size-1 matmuls: ~0 engine)
                        for jh in range(PCB // 2):
                            j = half * (PCB // 2) + jh
                            cb = panel * PCB + j
                            c0 = cb * 128
                            nc.tensor.matmul(
                                psum_cr[:, cb:cb + 1],
                                cache[:, ub, c0:c0 + 128], ones_hf[:],
                                start=False,
                                stop=(panel == PAN - 1 and ub == UB - 1),
                                skip_group_check=True)
                    nc.vector.tensor_copy(adjt[:, panel % 2, :, ub], pst[:])
                    # setup compute hooks (panel 0) and the lagged per-panel
                    # chain, emitted late enough that the AllReduce is
                    # already back (no queue-head stall)
                    if panel == 0:
                        if ub == 1:
                            setup_ownt()
                        elif ub == 2:
                            setup_eist(list(range(4)))
                        elif ub == 3:
                            setup_eist(list(range(4, UB)))
                        elif ub == 4:
                            setup_ag()

                # panel column sums complete -> AllReduce (latency hidden)
                csl = slice(panel * PCB, (panel + 1) * PCB)
                col_sb = small.tile([128, PCB], f32, tag="colsb")
                nc.vector.tensor_copy(col_sb[:], psum_cr[:, csl])
                nc.scalar.dma_start(col_in[panel][:], col_sb[:])
                if single:
                    nc.gpsimd.dma_start(col_out[panel][:], col_in[panel][:])
                else:
                    nc.gpsimd.collective_compute(
                        "AllReduce", mybir.AluOpType.add, replica_groups=groups,
                        ins=[col_in[panel].opt()], outs=[col_out[panel].opt()])
                cb2 = small.tile([128, PCB], f32, tag="cs2", name=f"cs2_{panel}")
                colsb2[panel] = cb2
                nc.gpsimd.dma_start(cb2[:], col_out[panel][:])

            # ---------------- tail
            emit_rowsums(*chunk_hist[-2])
            emit_rowsums(*chunk_hist[-1])

            # s_r and Xr (scale x0r in place)
            sqr = small.tile([128, UB], f32, tag="sqr2")
            nc.scalar.sqrt(sqr[:], psum_cr[:, CB:CB + UB])
            nc.vector.reciprocal(s_r[:], sqr[:])
            for ub in range(UB):
                nc.scalar.activation(xr[:, ub], xr[:, ub], AF.Copy,
                                     scale=s_r[:, ub:ub + 1])

            def emit_item_finish():
                """out_item = LReLU(s_r*(q0 + (q1*uown)@W2) + iown), done in
                transposed space: q0T/q1T = psum_qt[0:64]/[64:128]."""
                for h in range(2):
                    hsl = slice(4 * h, 4 * (h + 1))
                    g = fin.tile([128, 4, 128], f16, tag="g")
                    nc.vector.tensor_mul(g[64:128], psum_qt[64:128, hsl],
                                         uown_t[64:128, hsl])
                    # accumulate (q1*uown)@W2 directly onto q0T in PSUM
                    nc.tensor.matmul(psum_qt[0:64, hsl], w2_hi[64:128],
                                     g[64:128], start=False, stop=True,
                                     skip_group_check=True)
                    sh = fin.tile([64, 4, 128], f16, tag="g")
                    nc.vector.tensor_copy(sh[:], psum_qt[0:64, hsl])
                    tr_ps = ps_sm.tile([128, 4, D], f16, tag="sm")
                    for k in range(4):
                        ub = 4 * h + k
                        nc.tensor.transpose(tr_ps[:, k], sh[:, k],
                                            ident[0:64, 0:64])
                    for k in range(4):
                        ub = 4 * h + k
                        tb = small.tile([128, D], f32, tag="ft")
                        nc.vector.scalar_tensor_tensor(
                            tb[:], tr_ps[:, k], s_r[:, ub:ub + 1],
                            iown[:, ub], ALU.mult, ALU.add)
                        nc.vector.scalar_tensor_tensor(
                            out_stage[:, ub], tb[:], 0.2, tb[:],
                            ALU.mult, ALU.max)
                ui_view = upd_item.rearrange("(ub p) d -> p ub d", p=128)
                nc.gpsimd.dma_start(ui_view[:], out_stage[:])

            # P^T: stationary Xr[ub], moving natural cache; 512B-desc pairs.
            # Even subs feed p_in_a (each core's pairs 4g,4g+1 = ub 0-3),
            # odd subs feed p_in_b (ub 4-7); evens run first so the first
            # ReduceScatter + readback + user finish overlap the odd half.
            p_in_a = dram.tile([NCORES, 2, 128, 256], f16, name="p_in_a")
            p_in_b = dram.tile([NCORES, 2, 128, 256], f16, name="p_in_b")
            p_out_a = dram.tile([2, 128, 256], f16, name="p_out_a")
            p_out_b = dram.tile([2, 128, 256], f16, name="p_out_b")

            def emit_rs(p_in_t, p_out_t):
                if single:
                    nc.sync.dma_start(p_out_t[:], p_in_t[0])
                else:
                    nc.gpsimd.collective_compute(
                        "ReduceScatter", mybir.AluOpType.add,
                        replica_groups=groups,
                        ins=[p_in_t.opt()], outs=[p_out_t.opt()])

            pid = nc.vector.partition_id()
            uu_view = upd_user.rearrange("(ub p) d -> p ub d", p=128)

            def finish_user(h):
                """out_user = LReLU(s_c*(P0 + (P1*iown)@W2) + uown), half h,
                in transposed space directly from the ReduceScatter output."""
                hsl = slice(4 * h, 4 * (h + 1))
                p_out_t = p_out_a if h == 0 else p_out_b
                nc.sync.dma_start(
                    pt_sb[:, hsl].rearrange("p (b x) c -> p b (x c)", x=2),
                    p_out_t.rearrange("b d c -> d b c"))
                g2 = fin.tile([128, 4, 128], f16, tag="g")
                nc.vector.tensor_mul(g2[64:128],
                                     pt_sb[64:128, hsl],
                                     iown_t[64:128, hsl])
                ph2 = ps2k.tile([64, 4, 128], f32, tag="s2k")
                nc.tensor.matmul(ph2[:], w2_hi[64:128], g2[64:128],
                                 start=True, stop=True)
                sh2 = fin.tile([64, 4, 128], f16, tag="g")
                nc.vector.scalar_tensor_tensor(
                    sh2[:], pt_sb[0:64, hsl], 1.0, ph2[:], ALU.mult, ALU.add)
                tr2 = ps_sm.tile([128, 4, D], f16, tag="sm")
                for k in range(4):
                    nc.tensor.transpose(tr2[:, k], sh2[:, k],
                                        ident[0:64, 0:64])
                for k in range(4):
                    ub = 4 * h + k
                    if single:
                        sc_ap = s_c[:, ub:ub + 1]
                    else:
                        sc_ap = s_c[:, ds(pid * UB + ub, 1)]
                    t1 = small.tile([128, D], f32, tag="ft")
                    nc.vector.scalar_tensor_tensor(
                        t1[:], tr2[:, k], sc_ap, uown[:, ub],
                        ALU.mult, ALU.add)
                    nc.vector.scalar_tensor_tensor(
                        out_stage[:, ub], t1[:], 0.2, t1[:],
                        ALU.mult, ALU.max)
                    if k == 1:
                        nc.gpsimd.dma_start(
                            uu_view[:, 4 * h:4 * h + 2],
                            out_stage[:, 4 * h:4 * h + 2])
                nc.gpsimd.dma_start(uu_view[:, 4 * h + 2:4 * h + 4],
                                    out_stage[:, 4 * h + 2:4 * h + 4])

            sub_order = [2 * t for t in range(8)] + [2 * t + 1 for t in range(8)]
            for t, sub in enumerate(sub_order):
                pp = ps2k.tile([128, 512], f32, tag="s2k")
                for ub in range(UB):
                    nc.tensor.matmul(
                        pp[:], xr[:, ub], cache[:, ub, sub * 512:(sub + 1) * 512],
                        start=(ub == 0), stop=(ub == UB - 1),
                        skip_group_check=True)
                pcast = pstp.tile([128, 2, 256], f16, tag="pst")
                if t % 2:
                    nc.vector.tensor_copy(pcast[:], pp[:])
                else:
                    nc.scalar.activation(pcast[:], pp[:], AF.Copy)
                p_in_t = p_in_a if sub % 2 == 0 else p_in_b
                nc.sync.dma_start(
                    p_in_t[sub // 2].rearrange("b d c -> d b c"), pcast[:])
                if t == 0:
                    emit_sc_xc(PAN - 2)
                    emit_q(PAN - 2, 0, 4)
                elif t == 1:
                    emit_q(PAN - 2, 4, 8)
                elif t == 4:
                    emit_sc_xc(PAN - 1)
                elif t == 5:
                    emit_q(PAN - 1, 0, 4)
                elif t == 6:
                    emit_q(PAN - 1, 4, 8)
                elif t == 7:
                    emit_rs(p_in_a, p_out_a)
                elif t == 8:
                    emit_item_finish()
                elif t == 11:
                    finish_user(0)
            emit_rs(p_in_b, p_out_b)
            finish_user(1)

    nc.compile()
    return nc


def _get_nc(dbg=False):
    key = ("nc", dbg)
    if key not in _CACHE:
        _CACHE[key] = _build(dbg)
    return _CACHE[key]


def make_in_maps(user_embeddings, item_embeddings, adjacency_matrix, W1, W2):
    adj = np.ascontiguousarray(np.asarray(adjacency_matrix, dtype=np.float32))
    ue = np.ascontiguousarray(np.asarray(user_embeddings, dtype=np.float32))
    ie = np.ascontiguousarray(np.asarray(item_embeddings, dtype=np.float32))
    w1 = np.ascontiguousarray(np.asarray(W1, dtype=np.float32))
    w2 = np.ascontiguousarray(np.asarray(W2, dtype=np.float32))
    in_maps = []
    for k in range(NCORES):
        sl = slice(k * U, (k + 1) * U)
        in_maps.append({
            "adj": np.ascontiguousarray(adj[sl]),
            "user_own": np.ascontiguousarray(ue[sl]),
            "item_own": np.ascontiguousarray(ie[sl]),
            "w1": w1,
            "w2": w2,
        })
    return in_maps


def assemble(results):
    upd_user = np.concatenate([results[k]["upd_user"] for k in range(NCORES)], 0)
    upd_item = np.concatenate([results[k]["upd_item"] for k in range(NCORES)], 0)
    return upd_user, upd_item


def kernel(user_embeddings, item_embeddings, adjacency_matrix, W1, W2):
    import time
    import concourse.bass_utils as bass_utils
    nc = _get_nc()
    in_maps = make_in_maps(user_embeddings, item_embeddings, adjacency_matrix,
                           W1, W2)
    last = None
    for attempt in range(3):
        try:
            res = bass_utils.run_bass_kernel_spmd(
                nc, in_maps, core_ids=list(range(NCORES)), trace=False)
            return assemble(res.results)
        except Exception as e:  # transient NRT/axon failures
            last = e
            time.sleep(10)
    raise last
